# revision 1
# baseline (speedup 1.0000x reference)
"""GCN encoder (sigmoid gate + 2x GCNConv) on 8 Trainium2 NeuronCores.

Strategy (SPMD, one program on 8 cores):
  - Nodes are sharded contiguously across cores (12500 rows each); the small
    weight matrices are replicated.  Edges are assigned to the core owning
    their *destination* node; self loops are materialized as ordinary edges.
  - Per layer: dense feature-major chain on the owned rows produces
    g = (features @ W) rows, transposed to row-major fp16 and written to
    DRAM; a chunked AllGather replicates g across cores (4 chunks of
    25000 global rows so gathers can start before the whole table landed).
  - Sparse phase: edges are grouped by (dst window of 128 nodes, src block
    of 25000 rows).  Source rows are fetched with dma_gather (int16 indices
    into the 25000-row block) in batches of TQ tiles; each 128-edge tile is
    scatter-added via a one-hot matmul: S[e, slot] = (dstrel[e]==slot)*norm[e]
    built with a single fused tensor_scalar, then PSUM accumulates
    lhsT=gathered[128e,128f] @ rhs=S[128e,128slots] over all tiles of the
    window, yielding the feature-major output window directly.
  - norm = dinv[src]*dinv[dst] is folded into S, so no separate degree
    scaling pass exists on device; bias/relu are applied at PSUM flush.

The harness calls kernel(**inputs) with full-size inputs; everything below
is self-contained (no file reads).
"""

import math
import os

import numpy as np

import concourse.bacc as bacc
import concourse.bass as bass
import concourse.mybir as mybir
import concourse.tile as tile
from concourse import library_config
from concourse.bass_utils import run_bass_kernel_spmd
from concourse.masks import make_identity

F32 = mybir.dt.float32
F16 = mybir.dt.float16
I16 = mybir.dt.int16

N_CORES = 8
D = 128  # feature dim == hidden dim == partition count

LAST_RESULTS = None  # set by kernel(); lets a test harness grab the results
LAST_NC = None       # compiled Bass module of the last kernel() call
LAST_IN_MAPS = None  # per-core input dicts of the last kernel() call
LAST_META = None     # sharding metadata of the last kernel() call


# --------------------------------------------------------------------------
# host-side sharding / metadata
# --------------------------------------------------------------------------

class Meta:
    pass


def _prep(x, edge_index, gate_W, gate_b, W1, b1, W2, b2,
          n_cores=N_CORES, win=128, nblk=4, tq=32):
    """Shard inputs, group edges, build per-core device input dicts plus the
    (core-independent) program structure metadata."""
    x = np.asarray(x, np.float32)
    N, d = x.shape
    assert d == D
    src = np.asarray(edge_index[0]).astype(np.int64)
    dst = np.asarray(edge_index[1]).astype(np.int64)

    nloc = N // n_cores
    assert nloc * n_cores == N
    assert nloc % nblk == 0
    blk_sub = nloc // nblk          # rows each core contributes to a block
    blk_rows = blk_sub * n_cores    # rows of one gather table block
    assert blk_rows < 32768, "dma_gather idx is int16"
    nwin = math.ceil(nloc / win)

    deg = np.bincount(dst, minlength=N).astype(np.float64) + 1.0
    dinv = (1.0 / np.sqrt(deg)).astype(np.float32)

    loop = np.arange(N, dtype=np.int64)
    s_all = np.concatenate([src, loop])
    d_all = np.concatenate([dst, loop])
    norm_all = dinv[s_all] * dinv[d_all]

    # src -> (block, row inside block); block k holds rows [r*blk_sub,
    # (r+1)*blk_sub) of every core r's shard, concatenated in rank order
    # (matches chunked AllGather output layout).
    s_core = s_all // nloc
    s_rem = s_all % nloc
    s_blk = s_rem // blk_sub
    s_idx = (s_core * blk_sub + s_rem % blk_sub).astype(np.int64)

    e_core = d_all // nloc
    ld = d_all % nloc
    e_win = ld // win
    e_slot = ld % win

    # tiles per (window, block): max over cores so the program is identical
    key = ((e_core * nwin + e_win) * nblk + s_blk).astype(np.int64)
    cnt = np.bincount(key, minlength=n_cores * nwin * nblk)
    cnt = cnt.reshape(n_cores, nwin, nblk)
    T_wb = -(-cnt.max(axis=0) // 128)           # [nwin, nblk]

    tstart = np.zeros((nwin, nblk), np.int64)
    tstart[1:, :] = np.cumsum(T_wb[:-1, :], axis=0)
    blk_tiles = T_wb.sum(axis=0)                # [nblk]
    blk_off = np.concatenate([[0], np.cumsum(blk_tiles)])
    ntiles_tot = int(blk_off[-1])

    calls_blk = [int(math.ceil(blk_tiles[b] / tq)) for b in range(nblk)]
    icols_blk = [calls_blk[b] * tq * 8 for b in range(nblk)]
    icol_off = np.concatenate([[0], np.cumsum(icols_blk)]).astype(np.int64)
    icols_tot = int(icol_off[-1])

    m = Meta()
    m.n_cores, m.win, m.nblk, m.tq = n_cores, win, nblk, tq
    m.nloc, m.blk_sub, m.blk_rows, m.nwin = nloc, blk_sub, blk_rows, nwin
    m.T_wb, m.tstart = T_wb, tstart
    m.blk_tiles, m.blk_off = blk_tiles, blk_off
    m.calls_blk, m.icol_off = calls_blk, icol_off
    m.ntiles_tot, m.icols_tot = ntiles_tot, icols_tot
    # AllGather-overlap passes: consecutive block pairs
    m.passes = [list(range(p, min(p + 2, nblk))) for p in range(0, nblk, 2)]

    gw = np.asarray(gate_W, np.float16)
    w1 = np.asarray(W1, np.float16)
    w2 = np.asarray(W2, np.float16)
    gb = np.asarray(gate_b, np.float32).reshape(D, 1)
    b1r = np.asarray(b1, np.float32).reshape(D, 1)
    b2r = np.asarray(b2, np.float32).reshape(D, 1)

    in_maps = []
    for c in range(n_cores):
        sel = np.nonzero(e_core == c)[0]
        eb = s_blk[sel]
        ew = e_win[sel]
        order = np.lexsort((ew, eb))
        sel = sel[order]
        eb = eb[order]
        ew = ew[order]
        es = s_idx[sel]
        eslot = e_slot[sel]
        enorm = norm_all[sel]

        gkey = eb * nwin + ew
        group_start = np.searchsorted(gkey, np.arange(nblk * nwin))
        rank = np.arange(len(gkey)) - group_start[gkey]
        tg = rank // 128
        p = rank % 128
        bt = tstart[ew, eb] + tg                 # tile index inside block
        col = blk_off[eb] + bt                   # global meta column
        assert (tg < T_wb[ew, eb]).all()

        dstrel = np.zeros((128, ntiles_tot), np.float32)
        nrm = np.zeros((128, ntiles_tot), np.float32)
        dstrel[p, col] = eslot
        nrm[p, col] = enorm

        idx_cols = []
        for b in range(nblk):
            mask_b = eb == b
            flat = np.zeros(calls_blk[b] * tq * 128, np.int16)
            flat[(bt[mask_b] * 128 + p[mask_b])] = es[mask_b].astype(np.int16)
            for cidx in range(calls_blk[b]):
                v = flat[cidx * tq * 128:(cidx + 1) * tq * 128]
                idx_cols.append(v.reshape(tq * 8, 16).T)
        if idx_cols:
            idx16 = np.concatenate(idx_cols, axis=1)
        else:
            idx16 = np.zeros((16, 0), np.int16)
        assert idx16.shape == (16, icols_tot)
        idx16 = np.tile(idx16, (8, 1))

        xT = np.ascontiguousarray(x[c * nloc:(c + 1) * nloc].T
                                  .astype(np.float16))

        in_maps.append({
            "xT": xT,
            "gw": gw, "gbias": gb, "w1": w1, "b1": b1r, "w2": w2, "b2": b2r,
            "eidx": np.ascontiguousarray(idx16),
            "edst": dstrel,
            "enrm": nrm,
        })
    return in_maps, m


# --------------------------------------------------------------------------
# device program
# --------------------------------------------------------------------------

def _emit(tc, outs, ins, m, fake_collectives=False):
    """Emit the whole SPMD program inside a TileContext.

    fake_collectives=True replaces each AllGather with a local DRAM copy of
    equivalent dependency shape (for single-core TimelineSim timing runs)."""
    nc = tc.nc
    AG = mybir.AluOpType
    AF = mybir.ActivationFunctionType
    groups = [list(range(m.n_cores))]
    out_ap = outs["out"]

    def span(w):
        return min(m.win, m.nloc - w * m.win)

    with (
        tc.tile_pool(name="sb", bufs=1) as sb,
        tc.tile_pool(name="ps", bufs=1, space="PSUM") as ps,
        tc.tile_pool(name="dr", bufs=1, space="DRAM") as dr,
    ):
        nc.gpsimd.load_library(library_config.mlp)

        # ---- constants / params ------------------------------------------
        ident16 = sb.tile([128, 128], F16, tag="id16")
        make_identity(nc, ident16[:, :])
        ident32 = sb.tile([128, 128], F32, tag="id32")
        make_identity(nc, ident32[:, :])
        iota16 = sb.tile([128, 128], F16, tag="iota")
        nc.gpsimd.iota(iota16[:, :], pattern=[[1, 128]], base=0,
                       channel_multiplier=0,
                       allow_small_or_imprecise_dtypes=True)

        wgate = sb.tile([128, 128], F16, tag="wgate")
        nc.sync.dma_start(wgate[:, :], ins["gw"][:, :])
        w1sb = sb.tile([128, 128], F16, tag="w1sb")
        nc.sync.dma_start(w1sb[:, :], ins["w1"][:, :])
        w2sb = sb.tile([128, 128], F16, tag="w2sb")
        nc.sync.dma_start(w2sb[:, :], ins["w2"][:, :])
        gbias = sb.tile([128, 1], F32, tag="gbias")
        nc.sync.dma_start(gbias[:, :], ins["gbias"][:, :])
        b1sb = sb.tile([128, 1], F32, tag="b1sb")
        nc.sync.dma_start(b1sb[:, :], ins["b1"][:, :])
        b2sb = sb.tile([128, 1], F32, tag="b2sb")
        nc.sync.dma_start(b2sb[:, :], ins["b2"][:, :])

        # ---- resident edge metadata --------------------------------------
        dst_sb = sb.tile([128, max(m.ntiles_tot, 1)], F32, tag="dst_sb")
        nc.sync.dma_start(dst_sb[:, :m.ntiles_tot], ins["edst"][:, :])
        nrm_sb = sb.tile([128, max(m.ntiles_tot, 1)], F32, tag="nrm_sb")
        nc.sync.dma_start(nrm_sb[:, :m.ntiles_tot], ins["enrm"][:, :])

        h1T = sb.tile([128, m.nloc], F16, tag="h1T")
        accT = sb.tile([128, m.nloc], F32, tag="accT")

        # ---- DRAM scratch -------------------------------------------------
        g1_loc = dr.tile([m.nloc, 128], F16, tag="g1_loc")
        g2_loc = dr.tile([m.nloc, 128], F16, tag="g2_loc")
        g1_full = [dr.tile([m.blk_rows, 128], F16, tag=f"g1_full{k}",
                           name=f"g1_full{k}", addr_space="Shared")
                   for k in range(m.nblk)]
        g2_full = [dr.tile([m.blk_rows, 128], F16, tag=f"g2_full{k}",
                           name=f"g2_full{k}", addr_space="Shared")
                   for k in range(m.nblk)]

        # chunk index after which AllGather block k can fire
        ag_after = {}
        for k in range(m.nblk):
            cc = ((k + 1) * m.blk_sub - 1) // m.win
            ag_after.setdefault(cc, []).append(k)

        def dense_store(cc, src_f16_tile, cols, g_loc, g_full):
            tp = ps.tile([128, 128], F16, tag="tr", bufs=2)
            nc.tensor.transpose(tp[:cols, :], src_f16_tile[:, :cols],
                                ident16[:, :])
            trs = sb.tile([128, 128], F16, tag="trs", bufs=2)
            nc.scalar.copy(trs[:cols, :], tp[:cols, :])
            nc.sync.dma_start(g_loc[cc * m.win:cc * m.win + cols, :],
                              trs[:cols, :])
            for k in ag_after.get(cc, []):
                if fake_collectives:
                    nc.sync.dma_start(
                        g_full[k][:m.blk_sub, :],
                        g_loc[k * m.blk_sub:(k + 1) * m.blk_sub, :])
                else:
                    nc.gpsimd.collective_compute(
                        "AllGather", AG.bypass, replica_groups=groups,
                        ins=[g_loc[k * m.blk_sub:(k + 1) * m.blk_sub, :]],
                        outs=[g_full[k][:, :]],
                    )

        # ---- phase A: gate + W1 (feature-major), store g1 row-major ------
        for cc in range(m.nwin):
            cols = span(cc)
            xt = sb.tile([128, 128], F16, tag="xt", bufs=3)
            nc.sync.dma_start(xt[:, :cols],
                              ins["xT"][:, cc * m.win:cc * m.win + cols])
            pg = ps.tile([128, 128], F32, tag="dense", bufs=2)
            nc.tensor.matmul(pg[:, :cols], lhsT=wgate[:, :], rhs=xt[:, :cols],
                             start=True, stop=True)
            gt = sb.tile([128, 128], F16, tag="gate", bufs=2)
            nc.scalar.activation(gt[:, :cols], pg[:, :cols], AF.Sigmoid,
                                 bias=gbias[:, :])
            h0 = sb.tile([128, 128], F16, tag="h0", bufs=2)
            nc.vector.tensor_tensor(out=h0[:, :cols], in0=xt[:, :cols],
                                    in1=gt[:, :cols], op=AG.mult)
            p1 = ps.tile([128, 128], F32, tag="dense", bufs=2)
            nc.tensor.matmul(p1[:, :cols], lhsT=w1sb[:, :], rhs=h0[:, :cols],
                             start=True, stop=True)
            g1c = sb.tile([128, 128], F16, tag="gc", bufs=2)
            nc.scalar.copy(g1c[:, :cols], p1[:, :cols])
            dense_store(cc, g1c, cols, g1_loc, g1_full)

        # ---- sparse phase ------------------------------------------------
        # two passes over block pairs so AllGather chunk k+1 overlaps the
        # matmuls consuming chunk k; per-window partials accumulate in accT
        IGRP = 4  # idx cols loaded per DMA, in units of gather calls

        def spmm(g_full, flush):
            gbufs = {}
            idxbufs = {}

            def idx_slice(b, call):
                grp = call // IGRP
                if (b, grp) not in idxbufs:
                    ic0 = int(m.icol_off[b]) + grp * IGRP * m.tq * 8
                    cols = min(IGRP * m.tq * 8,
                               int(m.icol_off[b + 1]) - ic0)
                    buf = sb.tile([128, IGRP * m.tq * 8], I16, tag="idxb",
                                  bufs=4, name=f"idxb{b}_{grp}")
                    nc.sync.dma_start(buf[:, :cols],
                                      ins["eidx"][:, ic0:ic0 + cols])
                    idxbufs[(b, grp)] = buf
                off = (call % IGRP) * m.tq * 8
                return idxbufs[(b, grp)], off

            def ensure_gather(b, call):
                if (b, call) in gbufs:
                    return
                ntile = int(min(m.tq, m.blk_tiles[b] - call * m.tq))
                gbuf = sb.tile([128, m.tq, 128], F16, tag=f"gbuf{b % 2}",
                               bufs=2, name=f"gbuf{b}_{call}")
                nidx = ntile * 128
                ibuf, ioff = idx_slice(b, call)
                nc.gpsimd.dma_gather(
                    gbuf[:, :ntile, :], g_full[b][:, :],
                    ibuf[:, ioff:ioff + ntile * 8], nidx, nidx, 128,
                    single_packet=(nidx * 2 <= 4096))
                gbufs[(b, call)] = gbuf

            first_pass = {}
            for w in range(m.nwin):
                for p, blocks in enumerate(m.passes):
                    if sum(int(m.T_wb[w, b]) for b in blocks) > 0:
                        first_pass[w] = p
                        break
                assert w in first_pass

            for p, blocks in enumerate(m.passes):
                for w in range(m.nwin):
                    nmm = sum(int(m.T_wb[w, b]) for b in blocks)
                    if nmm == 0:
                        continue
                    cols = span(w)
                    for b in blocks:
                        if m.T_wb[w, b] == 0:
                            continue
                        t0 = int(m.tstart[w, b])
                        t1 = t0 + int(m.T_wb[w, b])
                        for call in range(t0 // m.tq, (t1 - 1) // m.tq + 1):
                            ensure_gather(b, call)
                    psw = ps.tile([128, 128], F32, tag="win", bufs=4)
                    k = 0
                    for b in blocks:
                        t0 = int(m.tstart[w, b])
                        for t in range(int(m.T_wb[w, b])):
                            bt = t0 + t
                            col = int(m.blk_off[b]) + bt
                            st = sb.tile([128, 128], F16, tag="st", bufs=6)
                            nc.vector.tensor_scalar(
                                st[:, :], iota16[:, :],
                                dst_sb[:, col:col + 1],
                                nrm_sb[:, col:col + 1],
                                op0=AG.is_equal, op1=AG.mult)
                            gbuf = gbufs[(b, bt // m.tq)]
                            nc.tensor.matmul(
                                psw[:, :], lhsT=gbuf[:, bt % m.tq, :],
                                rhs=st[:, :],
                                start=(k == 0), stop=(k == nmm - 1))
                            k += 1
                    aslice = accT[:, w * m.win:w * m.win + cols]
                    if p == first_pass[w]:
                        nc.scalar.copy(aslice, psw[:, :cols])
                    else:
                        nc.vector.tensor_tensor(out=aslice, in0=aslice,
                                                in1=psw[:, :cols], op=AG.add)
            for w in range(m.nwin):
                flush(w)

        def flush1(w):
            cols = span(w)
            nc.scalar.activation(h1T[:, w * m.win:w * m.win + cols],
                                 accT[:, w * m.win:w * m.win + cols],
                                 AF.Relu, bias=b1sb[:, :])

        spmm(g1_full, flush1)

        # ---- phase C: W2 on h1 (feature-major), store g2 row-major -------
        for cc in range(m.nwin):
            cols = span(cc)
            p2 = ps.tile([128, 128], F32, tag="dense", bufs=2)
            nc.tensor.matmul(p2[:, :cols], lhsT=w2sb[:, :],
                             rhs=h1T[:, cc * m.win:cc * m.win + cols],
                             start=True, stop=True)
            g2c = sb.tile([128, 128], F16, tag="gc", bufs=2)
            nc.scalar.copy(g2c[:, :cols], p2[:, :cols])
            dense_store(cc, g2c, cols, g2_loc, g2_full)

        # ---- phase D: second conv, add bias, transpose to row-major ------
        def flush2(w):
            cols = span(w)
            u2 = sb.tile([128, 128], F32, tag="u2", bufs=2)
            nc.vector.tensor_scalar(u2[:, :cols],
                                    accT[:, w * m.win:w * m.win + cols],
                                    b2sb[:, :], None, op0=AG.add)
            tp2 = ps.tile([128, 128], F32, tag="tr", bufs=2)
            nc.tensor.transpose(tp2[:cols, :], u2[:, :cols], ident32[:, :])
            tr2 = sb.tile([128, 128], F32, tag="tr2s", bufs=2)
            nc.scalar.copy(tr2[:cols, :], tp2[:cols, :])
            nc.sync.dma_start(out_ap[w * m.win:w * m.win + cols, :],
                              tr2[:cols, :])

        spmm(g2_full, flush2)


def declare_io(nc, m):
    ins = {
        "xT": nc.dram_tensor("xT", [D, m.nloc], F16, kind="ExternalInput").ap(),
        "gw": nc.dram_tensor("gw", [D, D], F16, kind="ExternalInput").ap(),
        "gbias": nc.dram_tensor("gbias", [D, 1], F32, kind="ExternalInput").ap(),
        "w1": nc.dram_tensor("w1", [D, D], F16, kind="ExternalInput").ap(),
        "b1": nc.dram_tensor("b1", [D, 1], F32, kind="ExternalInput").ap(),
        "w2": nc.dram_tensor("w2", [D, D], F16, kind="ExternalInput").ap(),
        "b2": nc.dram_tensor("b2", [D, 1], F32, kind="ExternalInput").ap(),
        "eidx": nc.dram_tensor("eidx", [128, max(m.icols_tot, 1)], I16,
                               kind="ExternalInput").ap(),
        "edst": nc.dram_tensor("edst", [128, max(m.ntiles_tot, 1)], F32,
                               kind="ExternalInput").ap(),
        "enrm": nc.dram_tensor("enrm", [128, max(m.ntiles_tot, 1)], F32,
                               kind="ExternalInput").ap(),
    }
    outs = {
        "out": nc.dram_tensor("out", [m.nloc, D], F32,
                              kind="ExternalOutput").ap(),
    }
    return ins, outs


def _build(m):
    nc = bacc.Bacc("TRN2", target_bir_lowering=False, debug=False,
                   enable_asserts=False, num_devices=m.n_cores)
    ins, outs = declare_io(nc, m)
    with tile.TileContext(nc) as tc:
        _emit(tc, outs, ins, m)
    nc.compile()
    return nc


def kernel(**inputs):
    global LAST_RESULTS, LAST_NC, LAST_IN_MAPS, LAST_META
    in_maps, m = _prep(**inputs)
    nc = _build(m)
    LAST_NC, LAST_IN_MAPS, LAST_META = nc, in_maps, m
    res = run_bass_kernel_spmd(
        nc, in_maps, core_ids=list(range(m.n_cores)), trace=False)
    LAST_RESULTS = res
    out = np.concatenate([res.results[c]["out"] for c in range(m.n_cores)],
                         axis=0)
    return np.ascontiguousarray(out.astype(np.float32))



# revision 19
# speedup vs baseline: 1.1756x; 1.1756x over previous
"""GCN encoder (sigmoid gate + 2x GCNConv) on 8 Trainium2 NeuronCores.

Strategy (SPMD, one program on 8 cores):
  - Nodes are sharded contiguously across cores (12500 rows each); the small
    weight matrices are replicated.  Edges are assigned to the core owning
    their *destination* node; self loops are materialized as ordinary edges.
  - Per layer: dense feature-major chain on the owned rows produces
    g = (features @ W) rows, transposed to row-major fp16 and written to
    DRAM; a chunked AllGather replicates g across cores (4 chunks of
    25000 global rows so gathers can start before the whole table landed).
  - Sparse phase: edges are grouped by (dst window of 128 nodes, src block
    of 25000 rows).  Source rows are fetched with dma_gather (int16 indices
    into the 25000-row block) in batches of TQ tiles; each 128-edge tile is
    scatter-added via a one-hot matmul: S[e, slot] = (dstrel[e]==slot)*norm[e]
    built with a single fused tensor_scalar, then PSUM accumulates
    lhsT=gathered[128e,128f] @ rhs=S[128e,128slots] over all tiles of the
    window, yielding the feature-major output window directly.
  - norm = dinv[src]*dinv[dst] is folded into S, so no separate degree
    scaling pass exists on device; bias/relu are applied at PSUM flush.

The harness calls kernel(**inputs) with full-size inputs; everything below
is self-contained (no file reads).
"""

import math
import os

import numpy as np

import concourse.bacc as bacc
import concourse.bass as bass
import concourse.mybir as mybir
import concourse.tile as tile
from concourse import library_config
from concourse.bass_utils import run_bass_kernel_spmd
from concourse.masks import make_identity

F32 = mybir.dt.float32
F16 = mybir.dt.float16
I16 = mybir.dt.int16

N_CORES = 8
D = 128  # feature dim == hidden dim == partition count

LAST_RESULTS = None  # set by kernel(); lets a test harness grab the results
LAST_NC = None       # compiled Bass module of the last kernel() call
LAST_IN_MAPS = None  # per-core input dicts of the last kernel() call
LAST_META = None     # sharding metadata of the last kernel() call


# --------------------------------------------------------------------------
# host-side sharding / metadata
# --------------------------------------------------------------------------

class Meta:
    pass


def _prep(x, edge_index, gate_W, gate_b, W1, b1, W2, b2,
          n_cores=N_CORES, win=128, nblk=4, tq=32):
    """Shard inputs, group edges, build per-core device input dicts plus the
    (core-independent) program structure metadata."""
    x = np.asarray(x, np.float32)
    N, d = x.shape
    assert d == D
    src = np.asarray(edge_index[0]).astype(np.int64)
    dst = np.asarray(edge_index[1]).astype(np.int64)

    nloc = N // n_cores
    assert nloc * n_cores == N
    assert nloc % nblk == 0
    blk_sub = nloc // nblk          # rows each core contributes to a block
    blk_rows = blk_sub * n_cores    # rows of one gather table block
    assert blk_rows < 32768, "dma_gather idx is int16"
    nwin = math.ceil(nloc / win)

    deg = np.bincount(dst, minlength=N).astype(np.float64) + 1.0
    dinv = (1.0 / np.sqrt(deg)).astype(np.float32)

    loop = np.arange(N, dtype=np.int64)
    s_all = np.concatenate([src, loop])
    d_all = np.concatenate([dst, loop])
    norm_all = dinv[s_all] * dinv[d_all]

    # src -> (block, row inside block); block k holds rows [r*blk_sub,
    # (r+1)*blk_sub) of every core r's shard, concatenated in rank order
    # (matches chunked AllGather output layout).
    s_core = s_all // nloc
    s_rem = s_all % nloc
    s_blk = s_rem // blk_sub
    s_idx = (s_core * blk_sub + s_rem % blk_sub).astype(np.int64)

    e_core = d_all // nloc
    ld = d_all % nloc
    e_win = ld // win
    e_slot = ld % win

    # tiles per (window, block): max over cores so the program is identical
    key = ((e_core * nwin + e_win) * nblk + s_blk).astype(np.int64)
    cnt = np.bincount(key, minlength=n_cores * nwin * nblk)
    cnt = cnt.reshape(n_cores, nwin, nblk)
    T_wb = -(-cnt.max(axis=0) // 128)           # [nwin, nblk]

    tstart = np.zeros((nwin, nblk), np.int64)
    tstart[1:, :] = np.cumsum(T_wb[:-1, :], axis=0)
    blk_tiles = T_wb.sum(axis=0)                # [nblk]
    blk_off = np.concatenate([[0], np.cumsum(blk_tiles)])
    ntiles_tot = int(blk_off[-1])

    calls_blk = [int(math.ceil(blk_tiles[b] / tq)) for b in range(nblk)]
    icols_blk = [calls_blk[b] * tq * 8 for b in range(nblk)]
    icol_off = np.concatenate([[0], np.cumsum(icols_blk)]).astype(np.int64)
    icols_tot = int(icol_off[-1])

    m = Meta()
    m.n_cores, m.win, m.nblk, m.tq = n_cores, win, nblk, tq
    m.nloc, m.blk_sub, m.blk_rows, m.nwin = nloc, blk_sub, blk_rows, nwin
    m.T_wb, m.tstart = T_wb, tstart
    m.blk_tiles, m.blk_off = blk_tiles, blk_off
    m.calls_blk, m.icol_off = calls_blk, icol_off
    m.ntiles_tot, m.icols_tot = ntiles_tot, icols_tot
    # AllGather-overlap passes: consecutive block pairs
    m.passes = [list(range(p, min(p + 2, nblk))) for p in range(0, nblk, 2)]

    gw = np.asarray(gate_W, np.float16)
    w1 = np.asarray(W1, np.float16)
    w2 = np.asarray(W2, np.float16)
    gb = np.asarray(gate_b, np.float32).reshape(D, 1)
    b1r = np.asarray(b1, np.float32).reshape(D, 1)
    b2r = np.asarray(b2, np.float32).reshape(D, 1)

    in_maps = []
    for c in range(n_cores):
        sel = np.nonzero(e_core == c)[0]
        eb = s_blk[sel]
        ew = e_win[sel]
        order = np.lexsort((ew, eb))
        sel = sel[order]
        eb = eb[order]
        ew = ew[order]
        es = s_idx[sel]
        eslot = e_slot[sel]
        enorm = norm_all[sel]

        gkey = eb * nwin + ew
        group_start = np.searchsorted(gkey, np.arange(nblk * nwin))
        rank = np.arange(len(gkey)) - group_start[gkey]
        tg = rank // 128
        p = rank % 128
        bt = tstart[ew, eb] + tg                 # tile index inside block
        col = blk_off[eb] + bt                   # global meta column
        assert (tg < T_wb[ew, eb]).all()

        dstrel = np.zeros((128, ntiles_tot), np.float32)
        nrm = np.zeros((128, ntiles_tot), np.float32)
        dstrel[p, col] = eslot
        nrm[p, col] = enorm

        idx_cols = []
        for b in range(nblk):
            mask_b = eb == b
            flat = np.zeros(calls_blk[b] * tq * 128, np.int16)
            flat[(bt[mask_b] * 128 + p[mask_b])] = es[mask_b].astype(np.int16)
            for cidx in range(calls_blk[b]):
                v = flat[cidx * tq * 128:(cidx + 1) * tq * 128]
                idx_cols.append(v.reshape(tq * 8, 16).T)
        if idx_cols:
            idx16 = np.concatenate(idx_cols, axis=1)
        else:
            idx16 = np.zeros((16, 0), np.int16)
        assert idx16.shape == (16, icols_tot)
        idx16 = np.tile(idx16, (8, 1))

        xT = np.ascontiguousarray(x[c * nloc:(c + 1) * nloc].T
                                  .astype(np.float16))

        in_maps.append({
            "xT": xT,
            "gw": gw, "gbias": gb, "w1": w1, "b1": b1r, "w2": w2, "b2": b2r,
            "eidx": np.ascontiguousarray(idx16),
            "edst": dstrel,
            "enrm": nrm,
            "chain": np.zeros((nloc, D), np.float32),
        })
    return in_maps, m


# --------------------------------------------------------------------------
# device program
# --------------------------------------------------------------------------

def _emit(tc, outs, ins, m, fake_collectives=False, skip_gather=False,
          skip_sbuild=False, gather_stub=False, iters=1):
    """Emit the whole SPMD program inside a TileContext.

    fake_collectives=True replaces each AllGather with a local DRAM copy of
    equivalent dependency shape (for single-core TimelineSim timing runs).
    skip_gather / skip_sbuild drop the dma_gather / S-build instructions
    (timing experiments only — results become garbage).
    iters>1 repeats the whole program in one NEFF (bench variants; shared
    DRAM scratch serializes iterations via data deps)."""
    nc = tc.nc
    AG = mybir.AluOpType
    AF = mybir.ActivationFunctionType
    groups = [list(range(m.n_cores))]
    out_ap = outs["out"]

    def span(w):
        return min(m.win, m.nloc - w * m.win)

    with (
        tc.tile_pool(name="sb", bufs=1) as sb,
        tc.tile_pool(name="ps", bufs=1, space="PSUM") as ps,
        tc.tile_pool(name="dr", bufs=1, space="DRAM") as dr,
    ):
        nc.gpsimd.load_library(library_config.mlp)

        # ---- constants / params ------------------------------------------
        ident16 = sb.tile([128, 128], F16, tag="id16")
        make_identity(nc, ident16[:, :])
        ident32 = sb.tile([128, 128], F32, tag="id32")
        make_identity(nc, ident32[:, :])
        iota16 = sb.tile([128, 128], F16, tag="iota")
        nc.gpsimd.iota(iota16[:, :], pattern=[[1, 128]], base=0,
                       channel_multiplier=0,
                       allow_small_or_imprecise_dtypes=True)

        wgate = sb.tile([128, 128], F16, tag="wgate")
        nc.sync.dma_start(wgate[:, :], ins["gw"][:, :])
        w1sb = sb.tile([128, 128], F16, tag="w1sb")
        nc.sync.dma_start(w1sb[:, :], ins["w1"][:, :])
        w2sb = sb.tile([128, 128], F16, tag="w2sb")
        nc.sync.dma_start(w2sb[:, :], ins["w2"][:, :])
        gbias = sb.tile([128, 1], F32, tag="gbias")
        nc.sync.dma_start(gbias[:, :], ins["gbias"][:, :])
        b1sb = sb.tile([128, 1], F32, tag="b1sb")
        nc.sync.dma_start(b1sb[:, :], ins["b1"][:, :])
        b2sb = sb.tile([128, 1], F32, tag="b2sb")
        nc.sync.dma_start(b2sb[:, :], ins["b2"][:, :])

        # ---- resident edge metadata --------------------------------------
        dst_sb = sb.tile([128, max(m.ntiles_tot, 1)], F32, tag="dst_sb")
        nc.sync.dma_start(dst_sb[:, :m.ntiles_tot], ins["edst"][:, :])
        nrm_sb = sb.tile([128, max(m.ntiles_tot, 1)], F32, tag="nrm_sb")
        nc.sync.dma_start(nrm_sb[:, :m.ntiles_tot], ins["enrm"][:, :])

        # ---- DRAM scratch -------------------------------------------------
        g1_loc = dr.tile([m.nloc, 128], F16, tag="g1_loc")
        g2_loc = dr.tile([m.nloc, 128], F16, tag="g2_loc")

        # chunk index after which AllGather block k can fire
        ag_after = {}
        for k in range(m.nblk):
            cc = ((k + 1) * m.blk_sub - 1) // m.win
            ag_after.setdefault(cc, []).append(k)

        def dense_store(cc, src_f16_tile, cols, g_loc, g_full):
            tp = ps.tile([128, 128], F16, tag="tr", bufs=2)
            nc.tensor.transpose(tp[:cols, :], src_f16_tile[:, :cols],
                                ident16[:, :])
            trs = sb.tile([128, 128], F16, tag="trs", bufs=2)
            nc.scalar.copy(trs[:cols, :], tp[:cols, :])
            nc.sync.dma_start(g_loc[cc * m.win:cc * m.win + cols, :],
                              trs[:cols, :])
            for k in ag_after.get(cc, []):
                if fake_collectives:
                    nc.sync.dma_start(
                        g_full[k][:m.blk_sub, :],
                        g_loc[k * m.blk_sub:(k + 1) * m.blk_sub, :])
                else:
                    nc.gpsimd.collective_compute(
                        "AllGather", AG.bypass, replica_groups=groups,
                        ins=[g_loc[k * m.blk_sub:(k + 1) * m.blk_sub, :]],
                        outs=[g_full[k][:, :]],
                    )

        # ---- sparse-phase tiling constants --------------------------------
        # two passes over block pairs so AllGather chunk k+1 overlaps the
        # matmuls consuming chunk k; per-window partials accumulate in accT
        IGRP = 4  # idx cols loaded per DMA, in units of gather calls

        def one_iter(_it):
            sfx = f"_it{_it}" if _it else ""
            g1_full = [dr.tile([m.blk_rows, 128], F16,
                               tag=f"g1_full{k}{sfx}",
                               name=f"g1_full{k}{sfx}", addr_space="Shared")
                       for k in range(m.nblk)]
            g2_full = [dr.tile([m.blk_rows, 128], F16,
                               tag=f"g2_full{k}{sfx}",
                               name=f"g2_full{k}{sfx}", addr_space="Shared")
                       for k in range(m.nblk)]

            # ---- phase A: gate + W1 (feature-major), store g1 row-major --
            for cc in range(m.nwin):
                cols = span(cc)
                xt = sb.tile([128, 128], F16, tag="xt", bufs=3)
                nc.sync.dma_start(xt[:, :cols],
                                  ins["xT"][:, cc * m.win:cc * m.win + cols])
                pg = ps.tile([128, 128], F32, tag="dense", bufs=2)
                nc.tensor.matmul(pg[:, :cols], lhsT=wgate[:, :],
                                 rhs=xt[:, :cols], start=True, stop=True)
                gt = sb.tile([128, 128], F16, tag="gate", bufs=2)
                nc.scalar.activation(gt[:, :cols], pg[:, :cols], AF.Sigmoid,
                                     bias=gbias[:, :])
                h0 = sb.tile([128, 128], F16, tag="h0", bufs=2)
                nc.vector.tensor_tensor(out=h0[:, :cols], in0=xt[:, :cols],
                                        in1=gt[:, :cols], op=AG.mult)
                p1 = ps.tile([128, 128], F32, tag="dense", bufs=2)
                nc.tensor.matmul(p1[:, :cols], lhsT=w1sb[:, :],
                                 rhs=h0[:, :cols], start=True, stop=True)
                g1c = sb.tile([128, 128], F16, tag="gc", bufs=2)
                nc.scalar.copy(g1c[:, :cols], p1[:, :cols])
                dense_store(cc, g1c, cols, g1_loc, g1_full)

            def spmm(g_full, finalize, lag=3):
                """Single pass over windows; all source blocks accumulate in
                one PSUM group per window; `finalize(w, psw)` runs `lag`
                windows behind the matmul front (so PE isn't stalled on the
                PSUM readout chain)."""
                gbufs = {}
                idxbufs = {}
                qctr = [0]

                def idx_slice(b, call):
                    grp = call // IGRP
                    if (b, grp) not in idxbufs:
                        ic0 = int(m.icol_off[b]) + grp * IGRP * m.tq * 8
                        cols = min(IGRP * m.tq * 8,
                                   int(m.icol_off[b + 1]) - ic0)
                        buf = sb.tile([128, IGRP * m.tq * 8], I16, tag="idxb",
                                      bufs=4, name=f"idxb{_it}_{b}_{grp}")
                        nc.sync.dma_start(buf[:, :cols],
                                          ins["eidx"][:, ic0:ic0 + cols])
                        idxbufs[(b, grp)] = buf
                    off = (call % IGRP) * m.tq * 8
                    return idxbufs[(b, grp)], off

                def ensure_gather(b, call):
                    if (b, call) in gbufs:
                        return
                    ntile = int(min(m.tq, m.blk_tiles[b] - call * m.tq))
                    gbuf = sb.tile([128, m.tq, 128], F16, tag=f"gbuf{b}",
                                   bufs=2, name=f"gbuf{_it}_{b}_{call}")
                    nidx = ntile * 128
                    if skip_gather:
                        gbufs[(b, call)] = gbuf
                        return
                    if gather_stub:
                        ntile, nidx = 1, 128
                    ibuf, ioff = idx_slice(b, call)
                    nc.gpsimd.dma_gather(
                        gbuf[:, :ntile, :], g_full[b][:, :],
                        ibuf[:, ioff:ioff + ntile * 8], nidx, nidx, 128,
                        single_packet=(nidx * 2 <= 4096),
                        queue_num=qctr[0] % 4)
                    qctr[0] += 1
                    gbufs[(b, call)] = gbuf

                pending = []
                for w in range(m.nwin):
                    nmm = sum(int(m.T_wb[w, b]) for b in range(m.nblk))
                    assert nmm > 0  # self loops guarantee every window
                    for b in range(m.nblk):
                        if m.T_wb[w, b] == 0:
                            continue
                        t0 = int(m.tstart[w, b])
                        t1 = t0 + int(m.T_wb[w, b])
                        for call in range(t0 // m.tq, (t1 - 1) // m.tq + 1):
                            ensure_gather(b, call)
                    psw = ps.tile([128, 128], F32, tag="win", bufs=4)
                    k = 0
                    for b in range(m.nblk):
                        t0 = int(m.tstart[w, b])
                        for t in range(int(m.T_wb[w, b])):
                            bt = t0 + t
                            col = int(m.blk_off[b]) + bt
                            st = sb.tile([128, 128], F16, tag="st", bufs=6)
                            if not skip_sbuild:
                                nc.vector.tensor_scalar(
                                    st[:, :], iota16[:, :],
                                    dst_sb[:, col:col + 1],
                                    nrm_sb[:, col:col + 1],
                                    op0=AG.is_equal, op1=AG.mult)
                            gbuf = gbufs[(b, bt // m.tq)]
                            nc.tensor.matmul(
                                psw[:, :], lhsT=gbuf[:, bt % m.tq, :],
                                rhs=st[:, :],
                                start=(k == 0), stop=(k == nmm - 1))
                            k += 1
                    pending.append((w, psw))
                    if len(pending) > lag:
                        finalize(*pending.pop(0))
                for w, psw in pending:
                    finalize(w, psw)

            def finalize1(w, psw):
                # relu(conv1 + b1) -> h1 window; W2 @ h1 -> g2 window; store
                # row-major + fire AllGather chunks as they complete
                cols = span(w)
                h1w = sb.tile([128, 128], F16, tag="h1w", bufs=4)
                nc.scalar.activation(h1w[:, :cols], psw[:, :cols],
                                     AF.Relu, bias=b1sb[:, :])
                p2 = ps.tile([128, 128], F32, tag="dense", bufs=2)
                nc.tensor.matmul(p2[:, :cols], lhsT=w2sb[:, :],
                                 rhs=h1w[:, :cols], start=True, stop=True)
                g2c = sb.tile([128, 128], F16, tag="gc", bufs=2)
                nc.scalar.copy(g2c[:, :cols], p2[:, :cols])
                dense_store(w, g2c, cols, g2_loc, g2_full)

            spmm(g1_full, finalize1)

            # ---- phase D: second conv, add bias, transpose to row-major --
            def finalize2(w, psw):
                cols = span(w)
                u2 = sb.tile([128, 128], F32, tag="u2", bufs=2)
                nc.vector.tensor_scalar(u2[:, :cols], psw[:, :cols],
                                        b2sb[:, :], None, op0=AG.add)
                tp2 = ps.tile([128, 128], F32, tag="tr", bufs=2)
                nc.tensor.transpose(tp2[:cols, :], u2[:, :cols],
                                    ident32[:, :])
                tr2 = sb.tile([128, 128], F32, tag="tr2s", bufs=2)
                nc.scalar.copy(tr2[:cols, :], tp2[:cols, :])
                nc.sync.dma_start(out_ap[w * m.win:w * m.win + cols, :],
                                  tr2[:cols, :])

            spmm(g2_full, finalize2)

        for _it in range(iters):
            one_iter(_it)


def declare_io(nc, m):
    ins = {
        "xT": nc.dram_tensor("xT", [D, m.nloc], F16, kind="ExternalInput").ap(),
        "gw": nc.dram_tensor("gw", [D, D], F16, kind="ExternalInput").ap(),
        "gbias": nc.dram_tensor("gbias", [D, 1], F32, kind="ExternalInput").ap(),
        "w1": nc.dram_tensor("w1", [D, D], F16, kind="ExternalInput").ap(),
        "b1": nc.dram_tensor("b1", [D, 1], F32, kind="ExternalInput").ap(),
        "w2": nc.dram_tensor("w2", [D, D], F16, kind="ExternalInput").ap(),
        "b2": nc.dram_tensor("b2", [D, 1], F32, kind="ExternalInput").ap(),
        "eidx": nc.dram_tensor("eidx", [128, max(m.icols_tot, 1)], I16,
                               kind="ExternalInput").ap(),
        "edst": nc.dram_tensor("edst", [128, max(m.ntiles_tot, 1)], F32,
                               kind="ExternalInput").ap(),
        "enrm": nc.dram_tensor("enrm", [128, max(m.ntiles_tot, 1)], F32,
                               kind="ExternalInput").ap(),
        # unused; lets a bench harness chain NEFF executions by feeding the
        # previous output in (pure PJRT data dependency, no device reads)
        "chain": nc.dram_tensor("chain", [m.nloc, D], F32,
                                kind="ExternalInput").ap(),
    }
    outs = {
        "out": nc.dram_tensor("out", [m.nloc, D], F32,
                              kind="ExternalOutput").ap(),
    }
    return ins, outs


def _build(m, **emit_knobs):
    nc = bacc.Bacc("TRN2", target_bir_lowering=False, debug=False,
                   enable_asserts=False, num_devices=m.n_cores,
                   num_swdge_queues=4)
    ins, outs = declare_io(nc, m)
    with tile.TileContext(nc) as tc:
        _emit(tc, outs, ins, m, **emit_knobs)
    nc.compile()
    return nc


def kernel(**inputs):
    global LAST_RESULTS, LAST_NC, LAST_IN_MAPS, LAST_META
    in_maps, m = _prep(**inputs)
    nc = _build(m)
    LAST_NC, LAST_IN_MAPS, LAST_META = nc, in_maps, m
    res = run_bass_kernel_spmd(
        nc, in_maps, core_ids=list(range(m.n_cores)), trace=False)
    LAST_RESULTS = res
    out = np.concatenate([res.results[c]["out"] for c in range(m.n_cores)],
                         axis=0)
    return np.ascontiguousarray(out.astype(np.float32))

